# revision 34
# baseline (speedup 1.0000x reference)
"""BertSelfAttention with gated prompt-prefix branch on 8 Trainium2 cores.

Sharding: data-parallel over batch (B=8 -> 1 batch element per core), no
collectives. Head-granular pipeline per core (12 stages), all bf16 matmuls
(fp8 DoubleRow measured 1 col/cycle on this HW - no gain over bf16):

  qT/kT = W @ hsT       [128, (chunk, s)] d-major, heads on 64-row halves
  scores_h = kT.T @ qT  [t, s], K=64 at tile rows (hp, 0), 512-col streams
  exp = exp(SCALE*score)  ACT, bf16 [t, s]; the 96 EXPs at ~1us each are
  the pacing backbone of the steady state
  ctx accumulates NATURAL [s, d]: stationary = exp[t-block, s-block 128],
  rhs = v_aug[t-block, 65] (col 64 = ones*e^mask -> denominator lands in
  psum col 64, PER-PARTITION in s). 65-col matmuls run ~30ns when the PE
  clock is ramped; single-bank accumulation uses pending-zero semantics
  (start=True only on the first matmul per bank).
  prefix branch identical from prompt-derived pkT/pv (tanh(gate) folded
  into pv on-device); prefix-ctx psum is a pool4 tile (sc7 at col 512).
  finish: DVE reciprocal + stride-0-broadcast tensor_mul +
  scalar_tensor_tensor, all per-partition; output written natural
  [1024, 768] f32 (no transpose anywhere).

Schedule notes (hard-won, keep): proj units emitted right after
scores_tci(h, 0) overlap the EXP-paced score stream (-13us); wq0/wk0
DMAs issued before the hsT chunks; PSUM pool4 bufs=3 shared by
scores/proj/prefix + ctx pool bufs=2 beat every split-pool variant;
deeper software pipelining (scores h+1 before ctx h) regresses.
"""

import numpy as np
import ml_dtypes

import concourse.bass as bass
import concourse.mybir as mybir
import concourse.tile as tile
from concourse.bass_utils import run_bass_kernel_spmd
from concourse.vector_clock import ScopedClock


class SplitDrainTileContext(tile.TileContext):
    """This walrus build rejects >2 sync waits on the kernel-tail Drain
    ("Too many sync wait commands"); split them across SP nops instead."""

    def _drain_and_barrier(self, tick_clock, wait_clock):
        probe = self.nc.sync.nop(nofuse=True, hint="drain_wait_split")
        wait_clock.add_sem_waits(
            probe.ins, ScopedClock({None: tick_clock.global_clock})
        )
        waits = list(probe.ins.sync_info.on_wait or [])
        if len(waits) > 1:
            probe.ins.sync_info.on_wait = waits[:1]
            for i in range(1, len(waits)):
                extra = self.nc.sync.nop(nofuse=True, hint="drain_wait_split")
                extra.ins.sync_info = mybir.SyncInfo(
                    on_wait=waits[i : i + 1], on_update=[]
                )
        drain_inst = self.nc.sync.drain()
        if drain_inst.ins.sync_info is not None:
            drain_inst.ins.sync_info.on_wait = []
        self.nc.all_engine_barrier()
        assert self.sems is not None
        popped = self.nc._tile_sem_poison_stack.pop()
        assert popped is self._sem_poison
        self.nc.clear_and_free_semaphores(list(self.sems.allocated().values()))
        self.nc.all_engine_barrier()

F32 = mybir.dt.float32
BF16 = mybir.dt.bfloat16
AF = mybir.ActivationFunctionType
ALU = mybir.AluOpType

H, DH, D = 12, 64, 768
S, AT, B = 1024, 64, 8
SCALE = 1.0 / np.sqrt(DH)
NC_D = D // 128  # 6 contraction chunks
NC_S = S // 128  # 8 sequence chunks
VW = H * (DH + 1)  # 780: v with per-head ones column

_CACHE = {}
LAST_RESULTS = None


def _split_sync_waits(nc, cap=1):
    """Walrus on this image allows very few sync-wait commands per
    instruction (tensor_scalar rejects 2). Hoist excess waits onto
    same-engine nops placed immediately before the instruction."""
    for bb in nc.main_func.blocks:
        cur = list(bb.instructions)
        out = []
        for inst in cur:
            si = inst.sync_info
            waits = list(si.on_wait) if si and si.on_wait else []
            if len(waits) > cap:
                for i in range(0, len(waits) - cap):
                    bi = nc.engines[inst.engine].nop(
                        nofuse=True, hint="wait_split")
                    popped = nc.cur_bb.bb.instructions.pop()
                    assert popped is bi.ins
                    bi.ins.sync_info = mybir.SyncInfo(
                        on_wait=waits[i : i + 1], on_update=[])
                    out.append(bi.ins)
                si.on_wait = waits[len(waits) - cap:]
            out.append(inst)
        bb.instructions[:] = out


def _build_nc():
    nc = bass.Bass()
    hsT = nc.dram_tensor("hsT", [D, S], BF16, kind="ExternalInput")
    wqT = nc.dram_tensor("wqT", [D, D], BF16, kind="ExternalInput")
    wkT = nc.dram_tensor("wkT", [D, D], BF16, kind="ExternalInput")
    wvT = nc.dram_tensor("wvT", [D, VW], BF16, kind="ExternalInput")
    bq = nc.dram_tensor("bq", [D, 1], F32, kind="ExternalInput")
    bk = nc.dram_tensor("bk", [D, 1], F32, kind="ExternalInput")
    bvaug = nc.dram_tensor("bvaug", [128, VW], F32, kind="ExternalInput")
    promptT = nc.dram_tensor("promptT", [D, AT], BF16, kind="ExternalInput")
    mask = nc.dram_tensor("mask", [S, 1], F32, kind="ExternalInput")
    gating = nc.dram_tensor("gating", [128, VW], F32, kind="ExternalInput")
    out_nat = nc.dram_tensor("out_nat", [S, D], F32, kind="ExternalOutput")

    with SplitDrainTileContext(nc) as tc:
        _emit(nc, tc, hsT, wqT, wkT, wvT, bq, bk, bvaug, promptT, mask,
              gating, out_nat)
    _split_sync_waits(nc)
    return nc


def _emit(nc, tc, hsT, wqT, wkT, wvT, bq, bk, bvaug, promptT, mask, gating,
          out_nat):
    from contextlib import ExitStack

    with ExitStack() as ctx:
        pers = ctx.enter_context(tc.tile_pool(name="pers", bufs=1))

        # ---- persistent SBUF ----
        hs_k = [pers.tile([128, S], BF16, tag=f"hs{k}", name=f"hs{k}")
                for k in range(NC_D)]
        wq_c = [pers.tile([128, D], BF16, tag=f"wq{c}", name=f"wq{c}")
                for c in range(NC_D)]
        wk_c = [pers.tile([128, D], BF16, tag=f"wk{c}", name=f"wk{c}")
                for c in range(NC_D)]
        wv_k = [pers.tile([128, VW], BF16, tag=f"wv{k}", name=f"wv{k}")
                for k in range(NC_D)]
        pT_sb = pers.tile([128, NC_D * AT], BF16, tag="pT")
        bq_sb = pers.tile([128, NC_D], F32, tag="bq")
        bk_sb = pers.tile([128, NC_D], F32, tag="bk")
        bvaug_sb = pers.tile([128, VW], F32, tag="bvaug")
        graw_sb = pers.tile([128, VW], F32, tag="graw")
        gbc_sb = pers.tile([128, VW], F32, tag="gbc")
        mask_sb = pers.tile([128, NC_S], F32, tag="mask")
        emask_sb = pers.tile([128, NC_S], F32, tag="emask")
        # bf16 q/k in d-major layout straight from the projection psum:
        # chunk c holds heads (2c, 2c+1) on partition halves
        qT_sb = pers.tile([128, NC_D * S], BF16, tag="qT")
        kT_sb = pers.tile([128, NC_D * S], BF16, tag="kT")
        pkT_sb = pers.tile([128, NC_D * AT], BF16, tag="pkT")
        v_sb = pers.tile([128, NC_S * VW], BF16, tag="v")
        pv_sb = pers.tile([128, VW], BF16, tag="pv")

        # ---- rotating SBUF pools ----
        exp_pool = ctx.enter_context(tc.tile_pool(name="expp", bufs=4))
        pexp_pool = exp_pool
        scratch = ctx.enter_context(tc.tile_pool(name="scr", bufs=3))
        vt_pool = out_pool = r_pool = scratch

        # ---- PSUM: pool4 = 3 tiles x [128,1024] (6 banks);
        #      ctx_pool = 2 tiles x [128,512] (2 banks) ----
        pool4 = ctx.enter_context(
            tc.tile_pool(name="p4", bufs=3, space="PSUM"))
        ctx_pool = ctx.enter_context(
            tc.tile_pool(name="ctxp", bufs=2, space="PSUM"))

        # ---- input DMAs, priority order: wq0/wk0 first (small), then
        # hsT chunks -- the first projection matmul needs only hs_k[0]
        # and accumulates in chunk-arrival order ----
        for c in (0,):
            nc.sync.dma_start(
                wq_c[c][:].rearrange("p (k n) -> p k n", n=128),
                wqT[:, c * 128:(c + 1) * 128].rearrange(
                    "(k p) n -> p k n", p=128))
            nc.sync.dma_start(
                wk_c[c][:].rearrange("p (k n) -> p k n", n=128),
                wkT[:, c * 128:(c + 1) * 128].rearrange(
                    "(k p) n -> p k n", p=128))
        for k in range(NC_D):
            nc.sync.dma_start(
                hs_k[k][:], hsT[k * 128:(k + 1) * 128, :])
        nc.sync.dma_start(bq_sb[:], bq.rearrange("(c p) 1 -> p c", p=128))
        nc.sync.dma_start(bk_sb[:], bk.rearrange("(c p) 1 -> p c", p=128))
        nc.sync.dma_start(mask_sb[:], mask.rearrange("(c p) 1 -> p c", p=128))
        nc.sync.dma_start(bvaug_sb[:], bvaug[:])
        nc.sync.dma_start(graw_sb[:], gating[:])
        nc.sync.dma_start(
            pT_sb[:].rearrange("p (k n) -> p k n", n=AT),
            promptT[:, :].rearrange("(k p) n -> p k n", p=128))
        for k in range(NC_D):
            nc.sync.dma_start(
                wv_k[k][:], wvT[k * 128:(k + 1) * 128, :])
        for c in range(1, NC_D):
            nc.sync.dma_start(
                wq_c[c][:].rearrange("p (k n) -> p k n", n=128),
                wqT[:, c * 128:(c + 1) * 128].rearrange(
                    "(k p) n -> p k n", p=128))
            nc.sync.dma_start(
                wk_c[c][:].rearrange("p (k n) -> p k n", n=128),
                wkT[:, c * 128:(c + 1) * 128].rearrange(
                    "(k p) n -> p k n", p=128))

        # ---- small precompute ----
        nc.scalar.activation(gbc_sb[:], graw_sb[:], AF.Tanh)
        ones_slots = gbc_sb[:, :].rearrange(
            "p (h e) -> p h e", h=H)[:, :, DH:DH + 1]
        nc.vector.memset(ones_slots, 1.0)
        nc.scalar.activation(emask_sb[:], mask_sb[:], AF.Exp)

        # ---- emission helpers ----
        def proj_qk(c, which):
            w_c, b_sb, dst = ((wq_c, bq_sb, qT_sb) if which == "q"
                              else (wk_c, bk_sb, kT_sb))
            ps = pool4.tile([128, S], F32, tag="p4", name=f"pqk_{c}_{which}")
            for kc in range(NC_D):
                lhsT = w_c[c][:, kc * 128:(kc + 1) * 128]
                for sb2 in range(2):
                    nc.tensor.matmul(
                        ps[:, sb2 * 512:(sb2 + 1) * 512], lhsT,
                        hs_k[kc][:, sb2 * 512:(sb2 + 1) * 512],
                        start=(kc == 0), stop=(kc == NC_D - 1))
            nc.vector.tensor_scalar_add(dst[:, c * S:(c + 1) * S], ps[:],
                                        b_sb[:, c:c + 1])

        def proj_pk(c):
            ps = pool4.tile([128, S], F32, tag="p4", name=f"ppk_{c}")
            for kc in range(NC_D):
                nc.tensor.matmul(
                    ps[:, 0:AT],
                    wk_c[c][:, kc * 128:(kc + 1) * 128],
                    pT_sb[:, kc * AT:(kc + 1) * AT],
                    start=(kc == 0), stop=(kc == NC_D - 1))
            nc.vector.tensor_scalar_add(pkT_sb[:, c * AT:(c + 1) * AT],
                                        ps[:, 0:AT], bk_sb[:, c:c + 1])

        def proj_v(sc):
            ps = pool4.tile([128, S], F32, tag="p4", name=f"pv_{sc}")
            for kc in range(NC_D):
                lhsT = hs_k[kc][:, sc * 128:(sc + 1) * 128]
                nc.tensor.matmul(ps[:, 0:512], lhsT, wv_k[kc][:, 0:512],
                                 start=(kc == 0), stop=(kc == NC_D - 1))
                nc.tensor.matmul(ps[:, 512:VW], lhsT, wv_k[kc][:, 512:VW],
                                 start=(kc == 0), stop=(kc == NC_D - 1))
            vt = vt_pool.tile([128, VW], F32, tag="vt", name=f"vt{sc}")
            nc.vector.tensor_add(vt[:], ps[:, 0:VW], bvaug_sb[:])
            nc.vector.tensor_scalar_mul(v_sb[:, sc * VW:(sc + 1) * VW],
                                        vt[:], emask_sb[:, sc:sc + 1])

        def proj_pv():
            ps = pool4.tile([128, S], F32, tag="p4", name="ppv")
            for kc in range(NC_D):
                lhsT = pT_sb[:, kc * AT:(kc + 1) * AT]
                nc.tensor.matmul(ps[0:AT, 0:512], lhsT, wv_k[kc][:, 0:512],
                                 start=(kc == 0), stop=(kc == NC_D - 1))
                nc.tensor.matmul(ps[0:AT, 512:VW], lhsT, wv_k[kc][:, 512:VW],
                                 start=(kc == 0), stop=(kc == NC_D - 1))
            pvt = vt_pool.tile([AT, VW], F32, tag="pvt", name="pvt")
            nc.vector.tensor_add(pvt[:], ps[0:AT, 0:VW], bvaug_sb[0:AT, :])
            nc.vector.tensor_mul(pv_sb[0:AT, :], pvt[:], gbc_sb[0:AT, :])
            nc.sync.dma_start(pv_sb[AT:128, :], pv_sb[0:AT, :])

        def scores_tci(h, tci, exp_h):
            c = h // 2
            hp = 64 * (h % 2)
            st = pool4.tile([128, S], F32, tag="p4",
                            name=f"st_{h}_{tci}")
            lhsT = kT_sb[hp:hp + 64,
                         c * S + tci * 128:c * S + (tci + 1) * 128]
            for sb2 in range(2):
                nc.tensor.matmul(
                    st[:, sb2 * 512:(sb2 + 1) * 512], lhsT,
                    qT_sb[hp:hp + 64,
                          c * S + sb2 * 512:c * S + (sb2 + 1) * 512],
                    start=True, stop=True, tile_position=(hp, 0))
            nc.scalar.activation(exp_h[:, tci * S:(tci + 1) * S],
                                 st[:], AF.Exp, scale=SCALE)

        def pfx_scores(c):
            ps = pool4.tile([128, S], F32, tag="p4", name=f"pfs_{c}")
            for g in range(2):
                h = 2 * c + g
                hp = 64 * g
                lhsT = pkT_sb[hp:hp + 64, c * AT:(c + 1) * AT]
                for sb2 in range(2):
                    nc.tensor.matmul(
                        ps[hp:hp + 64, sb2 * 512:(sb2 + 1) * 512],
                        lhsT,
                        qT_sb[hp:hp + 64,
                              c * S + sb2 * 512:c * S + (sb2 + 1) * 512],
                        start=True, stop=True, tile_position=(hp, hp))
            pexp = pexp_pool.tile([128, S], BF16, tag="pexp",
                                  name=f"pexp_{c}")
            nc.scalar.activation(pexp[:], ps[:], AF.Exp, scale=SCALE)
            return pexp

        def ctx_tci(h, tci, exp_h, ctxA, ctxB):
            for sc in range(NC_S):
                lhsT = exp_h[:, tci * S + sc * 128:tci * S + (sc + 1) * 128]
                rhs = v_sb[:, tci * VW + h * 65:tci * VW + h * 65 + 65]
                if sc < 7:
                    out = ctxA[:, sc * 65:(sc + 1) * 65]
                    st_fl = (tci == 0 and sc == 0)
                    sp_fl = (tci == NC_S - 1 and sc == 6)
                else:
                    out = ctxB[:, 0:65]
                    st_fl = (tci == 0)
                    sp_fl = (tci == NC_S - 1)
                nc.tensor.matmul(
                    out, lhsT, rhs, start=st_fl, stop=sp_fl,
                    skip_group_check=True)

        def pfx_ctx(h, pexp, pfxP):
            hp = 64 * (h % 2)
            for sc in range(NC_S):
                lhsT = pexp[hp:hp + 64, sc * 128:(sc + 1) * 128]
                rhs = pv_sb[hp:hp + 64, h * 65:h * 65 + 65]
                if sc < 7:
                    out = pfxP[:, sc * 65:(sc + 1) * 65]
                    st_fl, sp_fl = (sc == 0), (sc == 6)
                else:
                    out = pfxP[:, 512:577]
                    st_fl, sp_fl = True, True
                nc.tensor.matmul(out, lhsT, rhs, start=st_fl, stop=sp_fl,
                                 skip_group_check=True,
                                 tile_position=(hp, 0))

        def bcast7(r16, col):
            a = r16[:, col:col + 7]
            return bass.AP(a.tensor, a.offset, [a.ap[0], [1, 7], [0, 64]])

        def strided7(t, off):
            a = t[:]
            return bass.AP(a.tensor, a.offset + off, [a.ap[0], [65, 7]])

        def finish(h, ctxA, ctxB, pfxP):
            r16 = r_pool.tile([128, 16], F32, tag="r16", name=f"r16_{h}")
            cA = ctxA[:, 0:455].rearrange("p (a b) -> p a b", b=65)
            pA = pfxP[:, 0:455].rearrange("p (a b) -> p a b", b=65)
            nc.vector.reciprocal(r16[:, 0:7], strided7(ctxA, 64))
            nc.vector.reciprocal(r16[:, 7:8], ctxB[:, 64:65])
            nc.vector.reciprocal(r16[:, 8:15], strided7(pfxP, 64))
            nc.vector.reciprocal(r16[:, 15:16], pfxP[:, 576:577])
            outb = out_pool.tile([128, 512], F32, tag="ob", name=f"ob_{h}")
            o3 = outb[:].rearrange("p (a b) -> p a b", b=64)
            tmp = out_pool.tile([128, 448], F32, tag="tmp", name=f"tm_{h}")
            t3 = tmp[:].rearrange("p (a b) -> p a b", b=64)
            # prefix reads first: frees the pool4 slot pfxP occupies
            nc.vector.tensor_mul(t3[:, :, :], pA[:, :, 0:64], bcast7(r16, 8))
            nc.vector.tensor_scalar_mul(outb[:, 448:512], ctxB[:, 0:64],
                                        r16[:, 7:8])
            nc.vector.scalar_tensor_tensor(
                outb[:, 448:512], pfxP[:, 512:576], r16[:, 15:16],
                outb[:, 448:512], op0=ALU.mult, op1=ALU.add)
            nc.vector.tensor_mul(o3[:, 0:7, :], cA[:, :, 0:64],
                                 bcast7(r16, 0))
            nc.gpsimd.tensor_add(outb[:, 0:448], outb[:, 0:448], tmp[:])
            base = out_nat[:, :]
            dst = bass.AP(base.tensor, base.offset + h * 64,
                          [[D, 128], [128 * D, 8], [1, 64]])
            nc.sync.dma_start(dst, o3[:, :, :])

        # ---- master emission sequence ----
        proj_qk(0, "q")
        proj_qk(0, "k")
        proj_pk(0)
        proj_v(0)
        proj_v(1)

        # ctx of head h-1 interleaves into head h's score loop: its exp
        # stationaries are all ready (no EXP waits), the per-tci ACT slack
        # absorbs the 8 small ctx matmuls, and the pair-boundary PE blocks
        # (which starve ACT) shrink to prefix+finish only.
        pexp_cur = None
        hist = {}
        for h in range(H):
            c = h // 2
            exp_h = exp_pool.tile([128, NC_S * S], BF16, tag="exp",
                                  name=f"exp_{h}")
            if h >= 1:
                pe, pexp_p, _ = hist[h - 1]
                cA = ctx_pool.tile([128, 512], F32, tag="ctx",
                                   name=f"cA_{h - 1}")
                cB = ctx_pool.tile([128, 512], F32, tag="ctx",
                                   name=f"cB_{h - 1}")
                hist[h - 1] = (pe, pexp_p, (cA, cB))
            scores_tci(h, 0, exp_h)
            if h % 2 == 0 and c + 1 < NC_D:
                proj_qk(c + 1, "q")
            if h % 2 == 1 and c + 1 < NC_D:
                proj_qk(c + 1, "k")
            if h >= 1:
                ctx_tci(h - 1, 0, hist[h - 1][0], cA, cB)
            for tci in range(1, NC_S):
                scores_tci(h, tci, exp_h)
                if tci == 3 and h % 2 == 0:
                    pexp_cur = pfx_scores(c)
                if tci == 3 and h % 2 == 1 and c + 1 < NC_D:
                    proj_pk(c + 1)
                if h >= 1:
                    ctx_tci(h - 1, tci, hist[h - 1][0], cA, cB)
                if tci == 6 and h >= 1:
                    pfxP_p = pool4.tile([128, S], F32, tag="p4",
                                        name=f"pfxp_{h - 1}")
                    pfx_ctx(h - 1, hist[h - 1][1], pfxP_p)
            if h == 0:
                proj_pv()
                for sc in range(2, NC_S):
                    proj_v(sc)
            hist[h] = (exp_h, pexp_cur, None)
            if h >= 1:
                pe, pexp_p, (cA, cB) = hist.pop(h - 1)
                finish(h - 1, cA, cB, pfxP_p)
        # last head's ctx as a tail block
        exp_h, pexp_p, _ = hist[H - 1]
        ctxA = ctx_pool.tile([128, 512], F32, tag="ctx", name="cA_11")
        ctxB = ctx_pool.tile([128, 512], F32, tag="ctx", name="cB_11")
        for tci in range(NC_S):
            ctx_tci(H - 1, tci, exp_h, ctxA, ctxB)
        pfxP = pool4.tile([128, S], F32, tag="p4", name="pfxp_11")
        pfx_ctx(H - 1, pexp_p, pfxP)
        finish(H - 1, ctxA, ctxB, pfxP)


def _prep_inputs(hidden_states, prompt_tokens, gating_factor, attention_mask,
                 Wq, bq, Wk, bk, Wv, bv):
    bf = ml_dtypes.bfloat16
    hs = np.asarray(hidden_states, np.float32)
    mask = np.asarray(attention_mask, np.float32).reshape(B, S)
    wqT = np.ascontiguousarray(np.asarray(Wq, np.float32).T).astype(bf)
    wkT = np.ascontiguousarray(np.asarray(Wk, np.float32).T).astype(bf)
    # augmented WvT: [din, 780], col 65h+j = Wv.T[:, 64h+j], col 65h+64 = 0
    wvT_f = np.asarray(Wv, np.float32).T
    wvT_aug = np.zeros((D, VW), np.float32)
    idx = np.arange(D)
    aug_cols = (idx // DH) * (DH + 1) + (idx % DH)
    wvT_aug[:, aug_cols] = wvT_f
    wvT_aug = wvT_aug.astype(bf)
    bq_c = np.asarray(bq, np.float32).reshape(D, 1)
    bk_c = np.asarray(bk, np.float32).reshape(D, 1)
    bv_aug = np.zeros(VW, np.float32)
    bv_aug[aug_cols] = np.asarray(bv, np.float32)
    bv_aug[DH::DH + 1] = 1.0
    bvaug_bc = np.ascontiguousarray(
        np.broadcast_to(bv_aug, (128, VW)), np.float32)
    pT = np.ascontiguousarray(
        np.asarray(prompt_tokens, np.float32)[0].T).astype(bf)
    gat_row = np.repeat(
        np.asarray(gating_factor, np.float32).reshape(H), DH + 1)
    gat = np.ascontiguousarray(
        np.broadcast_to(gat_row, (128, VW)), np.float32)

    shared = dict(wqT=wqT, wkT=wkT, wvT=wvT_aug, bq=bq_c, bk=bk_c,
                  bvaug=bvaug_bc, promptT=pT, gating=gat)
    in_maps = []
    for b in range(B):
        m = dict(shared)
        m["hsT"] = np.ascontiguousarray(hs[b].T).astype(bf)
        m["mask"] = np.ascontiguousarray(mask[b].reshape(S, 1))
        in_maps.append(m)
    return in_maps


def kernel(**inputs):
    global LAST_RESULTS
    if "nc" not in _CACHE:
        _CACHE["nc"] = _build_nc()
    nc = _CACHE["nc"]
    in_maps = _prep_inputs(**inputs)
    res = None
    for attempt in range(3):
        try:
            res = run_bass_kernel_spmd(nc, in_maps, list(range(B)))
            break
        except ModuleNotFoundError:
            import os

            os.environ["BASS_NEVER_TRACE"] = "1"
            if attempt == 2:
                raise
        except Exception:
            if attempt == 2:
                raise
    LAST_RESULTS = res
    out = np.empty((B, S, D), np.float32)
    for b in range(B):
        out[b] = res.results[b]["out_nat"]
    return out
